# revision 2
# baseline (speedup 1.0000x reference)
"""AERIALAgent distributed Trainium2 kernel (8 NeuronCores).

Strategy (row/data parallel over the agent axis, per spec sharding hint):
  - Each core owns 1024 of the 8192 agents.
  - belief projection bp = beliefs @ Wb + bb is computed locally in
    transposed layout [64, 1024] (bf16) and all-gathered (tiny, 128KB/core).
  - The 8192x8192 attention is done flash-style per core: scores are
    computed TRANSPOSED (ST[j, a] tiles) so softmax-weighted accumulation
    (PV) needs no on-device transposes of the probability matrix.
  - Softmax runs without max subtraction (logits are in [-3, 6]) and the
    diagonal mask is applied algebraically: the diag term exp(|bp_a|^2/8)
    is subtracted from both the numerator (context) and denominator after
    the PV accumulation, instead of masking per-tile.
  - GRU runs fully in transposed [hidden, agent] layout with composite
    weights (Wg @ Wi*) folded on the host; sigmoid is computed via tanh
    so a single ACT table set (exp/tanh/relu/copy) serves the kernel.
  - All device matmuls are bf16 (f32 PSUM accumulation) except the tiny
    logits head and broadcast helpers which are f32.
  - Outputs come back transposed; the host transposes them back.

The harness contract: kernel(**inputs) -> (logits [8192,6], new_beliefs
[8192,128]) with full (unsharded) numpy inputs, matching reference.py.
"""

import numpy as np
import ml_dtypes

import concourse.bass as bass
import concourse.bacc as bacc
import concourse.mybir as mybir
import concourse.tile as tile
from concourse.bass_utils import run_bass_kernel_spmd

BF16_NP = ml_dtypes.bfloat16
F32 = mybir.dt.float32
BF16 = mybir.dt.bfloat16
AF = mybir.ActivationFunctionType
ALU = mybir.AluOpType

CORES = 8
A = 8192           # total agents
AL = A // CORES    # 1024 agents per core
OBS = 520
OBSP = 640         # obs dim padded to 5 * 128
KC = OBSP // 128   # 5 contraction chunks for the obs matmul
E = 64             # embed
H = 128            # hidden
NACT = 6
JT = A // 128      # 64 key tiles
NSB = AL // 512    # 2 superblocks of 512 query agents
JSTR = 66          # bp_aug per-tile column stride (64 data + 1 ones + 1 pad)

# wpack column offsets (bf16 [128, WCOLS])
W1_O = 0              # 5 chunks of 64
W2_O = 320
WB_O = 384
WGR_O = 448
WGZ_O = 576
WGN_O = 704
WHR_O = 832
WHZ_O = 960
WHN_O = 1088          # 0.5 * Whn
ID_O = 1216           # eye(128)
BELB_O = 1344         # beliefs.T bf16 [128, 1024]
WCOLS = BELB_O + AL

# brow column offsets (bf16 [1, BCOLS])
B1_O = 0
B2_O = 64
BB_O = 128
BR_O = 192            # bhr + bg @ Wir
BZ_O = 320            # bhz + bg @ Wiz
BHN_O = 448           # 0.5 * bhn
BGN_O = 576           # bg @ Win
BOUT_O = 704
BCOLS = 712


def _build_nc():
    nc = bacc.Bacc(
        "TRN2",
        target_bir_lowering=False,
        debug=False,
        num_devices=CORES,
    )

    obsT_d = nc.dram_tensor("obsT", [128, KC * AL], BF16, kind="ExternalInput")
    belT_d = nc.dram_tensor("belT", [H, AL], F32, kind="ExternalInput")
    wpack_d = nc.dram_tensor("wpack", [128, WCOLS], BF16, kind="ExternalInput")
    brow_d = nc.dram_tensor("brow", [1, BCOLS], BF16, kind="ExternalInput")
    wout_d = nc.dram_tensor("wout", [H, NACT], F32, kind="ExternalInput")

    out_nbT_d = nc.dram_tensor("out_nbT", [H, AL], F32, kind="ExternalOutput")
    out_lg_d = nc.dram_tensor(
        "out_lg", [128, (AL // 128) * NACT], F32, kind="ExternalOutput"
    )

    cc_in_d = nc.dram_tensor("cc_in", [E, AL], BF16)
    cc_out_d = nc.dram_tensor("cc_out", [CORES * E, AL], BF16, addr_space="Shared")

    with tile.TileContext(nc) as tc:
        with (
            tc.tile_pool(name="const", bufs=1) as const,
            tc.tile_pool(name="work", bufs=2) as work,
            tc.tile_pool(name="expp", bufs=4) as expp,
            tc.tile_pool(name="pst", bufs=2, space=bass.MemorySpace.PSUM) as pst,
            tc.tile_pool(name="pctx", bufs=2, space=bass.MemorySpace.PSUM) as pctx,
            tc.tile_pool(name="pmisc", bufs=2, space=bass.MemorySpace.PSUM) as pmisc,
        ):
            # ---------------- constants / inputs ----------------
            w_sb = const.tile([128, WCOLS], BF16, name="w_sb")
            obsT_sb = const.tile([128, KC * AL], BF16, name="obsT_sb")
            belT_sb = const.tile([H, AL], F32, name="belT_sb")
            brow_sb = const.tile([1, BCOLS], BF16, name="brow_sb")
            wout_sb = const.tile([H, NACT], F32, name="wout_sb")

            nc.sync.dma_start(w_sb[:], wpack_d[:])
            nc.sync.dma_start(obsT_sb[:], obsT_d[:])
            nc.sync.dma_start(belT_sb[:], belT_d[:])
            nc.sync.dma_start(brow_sb[:], brow_d[:])
            nc.sync.dma_start(wout_sb[:], wout_d[:])

            ones_sb = const.tile([1, 512], BF16, name="ones_sb")
            onesf_sb = const.tile([1, E], F32, name="onesf_sb")
            onesc_sb = const.tile([E, 1], BF16, name="onesc_sb")
            nc.vector.memset(ones_sb[:], 1.0)
            nc.vector.memset(onesf_sb[:], 1.0)
            nc.vector.memset(onesc_sb[:], 1.0)

            bpTl_sb = const.tile([E, AL], BF16, name="bpTl_sb")
            sq_sb = const.tile([E, AL], BF16, name="sq_sb")
            expd_sb = const.tile([1, AL], F32, name="expd_sb")
            bpT_sb = const.tile([E, A], BF16, name="bpT_sb")
            bp_aug_sb = const.tile([128, JT * JSTR], BF16, name="bp_aug_sb")
            h1T_sb = const.tile([E, AL], BF16, name="h1T_sb")
            concatT_sb = const.tile([H, AL], BF16, name="concatT_sb")
            nbT_sb = const.tile([H, AL], F32, name="nbT_sb")
            lg_sb = const.tile([128, (AL // 128) * NACT], F32, name="lg_sb")
            den_sb = const.tile([1, AL], F32, name="den_sb")
            recip_sb = const.tile([1, AL], F32, name="recip_sb")

            # -------- local belief projection bpT = (Wb.T @ beliefs.T) + bb --------
            for ch in range(2):
                cs = slice(ch * 512, (ch + 1) * 512)
                pb = pmisc.tile([128, 512], F32, name="pm")
                nc.tensor.matmul(
                    pb[0:E, :],
                    w_sb[:, WB_O : WB_O + E],
                    w_sb[:, BELB_O + ch * 512 : BELB_O + (ch + 1) * 512],
                    start=True,
                    stop=False,
                )
                nc.tensor.matmul(
                    pb[0:E, :],
                    brow_sb[:, BB_O : BB_O + E],
                    ones_sb[:, 0:512],
                    start=False,
                    stop=True,
                )
                nc.scalar.activation(bpTl_sb[:, cs], pb[0:E, :], AF.Copy)

            # self-dot -> exp(diag) for the diagonal-mask correction
            nc.vector.tensor_tensor(sq_sb[:], bpTl_sb[:], bpTl_sb[:], ALU.mult)
            for ch in range(2):
                cs = slice(ch * 512, (ch + 1) * 512)
                psd = pmisc.tile([128, 512], F32, name="pm")
                nc.tensor.matmul(
                    psd[0:1, :], onesc_sb[:], sq_sb[:, cs], start=True, stop=True
                )
                nc.scalar.activation(expd_sb[:, cs], psd[0:1, :], AF.Exp, scale=0.125)

            # -------- all-gather bpT across the 8 cores --------
            nc.gpsimd.dma_start(cc_in_d[:], bpTl_sb[:])
            nc.gpsimd.collective_compute(
                "AllGather",
                ALU.bypass,
                replica_groups=[list(range(CORES))],
                ins=[cc_in_d.ap().opt()],
                outs=[cc_out_d.ap().opt()],
            )
            # cc_out is [(r e), a]; load as [e, (r a)]
            nc.sync.dma_start(
                bpT_sb[:].rearrange("e (r a) -> e r a", r=CORES),
                cc_out_d.ap().rearrange("(r e) a -> e r a", r=CORES),
            )

            # -------- bp_aug: per key-tile [128, 64+1] = [bp rows | ones] --------
            ones_col = bp_aug_sb[:].rearrange("p (j c) -> p j c", c=JSTR)[:, :, E : E + 1]
            nc.vector.memset(ones_col, 1.0)
            for g in range(8):
                pt = pmisc.tile([128, 512], BF16, name="ptr", tag="pm")
                for i in range(8):
                    jt = g * 8 + i
                    nc.tensor.transpose(
                        pt[:, i * 64 : (i + 1) * 64],
                        bpT_sb[:, jt * 128 : (jt + 1) * 128],
                        w_sb[0:E, ID_O : ID_O + E],
                    )
                dst = bp_aug_sb[
                    :, g * 8 * JSTR : (g + 1) * 8 * JSTR
                ].rearrange("p (j c) -> p j c", c=JSTR)[:, :, 0:E]
                nc.vector.tensor_copy(
                    dst, pt[:].rearrange("p (j c) -> p j c", c=64)
                )

            # -------- obs MLP (transposed): h2T = relu(W2.T relu(W1.T obsT + b1) + b2) --------
            for ch in range(2):
                cs = slice(ch * 512, (ch + 1) * 512)
                p1 = pmisc.tile([128, 512], F32, name="pm")
                for kc in range(KC):
                    nc.tensor.matmul(
                        p1[0:E, :],
                        w_sb[:, W1_O + kc * E : W1_O + (kc + 1) * E],
                        obsT_sb[:, kc * AL + ch * 512 : kc * AL + (ch + 1) * 512],
                        start=(kc == 0),
                        stop=False,
                    )
                nc.tensor.matmul(
                    p1[0:E, :],
                    brow_sb[:, B1_O : B1_O + E],
                    ones_sb[:, 0:512],
                    start=False,
                    stop=True,
                )
                nc.scalar.activation(h1T_sb[:, cs], p1[0:E, :], AF.Relu)
            for ch in range(2):
                cs = slice(ch * 512, (ch + 1) * 512)
                p2 = pmisc.tile([128, 512], F32, name="pm")
                nc.tensor.matmul(
                    p2[0:E, :],
                    w_sb[0:E, W2_O : W2_O + E],
                    h1T_sb[:, cs],
                    start=True,
                    stop=False,
                )
                nc.tensor.matmul(
                    p2[0:E, :],
                    brow_sb[:, B2_O : B2_O + E],
                    ones_sb[:, 0:512],
                    start=False,
                    stop=True,
                )
                nc.scalar.activation(concatT_sb[0:E, cs], p2[0:E, :], AF.Relu)

            # -------- attention + context fix + GRU, per 512-agent superblock --------
            for sb in range(NSB):
                cs = slice(sb * 512, (sb + 1) * 512)
                ctx = pctx.tile([128, 512], F32, name="ctx")
                for g in range(JT // 2):
                    st = pst.tile([128, 1024], F32, name="st")
                    for i in range(2):
                        jt = g * 2 + i
                        nc.tensor.matmul(
                            st[:, i * 512 : (i + 1) * 512],
                            bpT_sb[:, jt * 128 : (jt + 1) * 128],
                            bpTl_sb[:, cs],
                            start=True,
                            stop=True,
                        )
                    ex = expp.tile([128, 1024], BF16, name="ex")
                    nc.scalar.activation(ex[:], st[:], AF.Exp, scale=0.125)
                    for i in range(2):
                        jt = g * 2 + i
                        nc.tensor.matmul(
                            ctx[0 : E + 1, :],
                            bp_aug_sb[:, jt * JSTR : jt * JSTR + E + 1],
                            ex[:, i * 512 : (i + 1) * 512],
                            start=(jt == 0),
                            stop=(jt == JT - 1),
                        )

                # remove diagonal; normalize; write context into concatT rows 64..127
                nc.vector.tensor_tensor(
                    den_sb[:, cs], ctx[E : E + 1, :], expd_sb[:, cs], ALU.subtract
                )
                nc.vector.reciprocal(recip_sb[:, cs], den_sb[:, cs])
                edb = pmisc.tile([128, 512], F32, name="pm")
                nc.tensor.matmul(
                    edb[0:E, :], onesf_sb[:], expd_sb[:, cs], start=True, stop=True
                )
                rb = pmisc.tile([128, 512], F32, name="pm")
                nc.tensor.matmul(
                    rb[0:E, :], onesf_sb[:], recip_sb[:, cs], start=True, stop=True
                )
                m_sb = work.tile([E, 512], F32, name="m_sb")
                nc.vector.tensor_tensor(m_sb[:], bpTl_sb[:, cs], edb[0:E, :], ALU.mult)
                t_sb = work.tile([E, 512], F32, name="t_sb")
                nc.vector.tensor_tensor(t_sb[:], ctx[0:E, :], m_sb[:], ALU.subtract)
                nc.vector.tensor_tensor(
                    concatT_sb[E:H, cs], t_sb[:], rb[0:E, :], ALU.mult
                )

                # ---- GRU for this 512-agent chunk (transposed layout) ----
                def gate_psum(wofs, bofs, extra_w=None):
                    p = pmisc.tile([128, 512], F32, name="pm")
                    nc.tensor.matmul(
                        p[:],
                        w_sb[:, wofs : wofs + H],
                        concatT_sb[:, cs] if extra_w is None else extra_w,
                        start=True,
                        stop=False,
                    )
                    return p

                belb_rhs = w_sb[:, BELB_O + sb * 512 : BELB_O + (sb + 1) * 512]

                # r gate
                pr = pmisc.tile([128, 512], F32, name="pm")
                nc.tensor.matmul(pr[:], w_sb[:, WGR_O : WGR_O + H], concatT_sb[:, cs],
                                 start=True, stop=False)
                nc.tensor.matmul(pr[:], w_sb[:, WHR_O : WHR_O + H], belb_rhs,
                                 start=False, stop=False)
                nc.tensor.matmul(pr[:], brow_sb[:, BR_O : BR_O + H], ones_sb[:, 0:512],
                                 start=False, stop=True)
                tr_sb = work.tile([H, 512], F32, name="tr_sb")
                nc.scalar.activation(tr_sb[:], pr[:], AF.Tanh, scale=0.5)

                # z gate
                pz = pmisc.tile([128, 512], F32, name="pm")
                nc.tensor.matmul(pz[:], w_sb[:, WGZ_O : WGZ_O + H], concatT_sb[:, cs],
                                 start=True, stop=False)
                nc.tensor.matmul(pz[:], w_sb[:, WHZ_O : WHZ_O + H], belb_rhs,
                                 start=False, stop=False)
                nc.tensor.matmul(pz[:], brow_sb[:, BZ_O : BZ_O + H], ones_sb[:, 0:512],
                                 start=False, stop=True)
                tz_sb = work.tile([H, 512], F32, name="tz_sb")
                nc.scalar.activation(tz_sb[:], pz[:], AF.Tanh, scale=0.5)

                # candidate: n = tanh(concat@Wgn + bgn + r*(bel@Whn + bhn))
                pu = pmisc.tile([128, 512], F32, name="pm")
                nc.tensor.matmul(pu[:], w_sb[:, WHN_O : WHN_O + H], belb_rhs,
                                 start=True, stop=False)
                nc.tensor.matmul(pu[:], brow_sb[:, BHN_O : BHN_O + H], ones_sb[:, 0:512],
                                 start=False, stop=True)
                pn = pmisc.tile([128, 512], F32, name="pm")
                nc.tensor.matmul(pn[:], w_sb[:, WGN_O : WGN_O + H], concatT_sb[:, cs],
                                 start=True, stop=False)
                nc.tensor.matmul(pn[:], brow_sb[:, BGN_O : BGN_O + H], ones_sb[:, 0:512],
                                 start=False, stop=True)
                # v = (tr + 1) * u'   (u' = 0.5*(bel@Whn + bhn))
                v_sb = work.tile([H, 512], F32, name="v_sb")
                nc.vector.scalar_tensor_tensor(
                    v_sb[:], tr_sb[:], 1.0, pu[:], ALU.add, ALU.mult
                )
                t2_sb = work.tile([H, 512], F32, name="t2_sb")
                nc.vector.tensor_tensor(t2_sb[:], v_sb[:], pn[:], ALU.add)
                n_sb = work.tile([H, 512], F32, name="n_sb")
                nc.scalar.activation(n_sb[:], t2_sb[:], AF.Tanh)

                # new_beliefs = n + 0.5*(tz + 1)*(beliefs - n)
                d_sb = work.tile([H, 512], F32, name="d_sb")
                nc.vector.tensor_tensor(d_sb[:], belT_sb[:, cs], n_sb[:], ALU.subtract)
                w2_sb = work.tile([H, 512], F32, name="w2_sb")
                nc.vector.scalar_tensor_tensor(
                    w2_sb[:], tz_sb[:], 1.0, d_sb[:], ALU.add, ALU.mult
                )
                nc.vector.scalar_tensor_tensor(
                    nbT_sb[:, cs], w2_sb[:], 0.5, n_sb[:], ALU.mult, ALU.add
                )

                # logits for the 4 query blocks in this chunk
                plg = pmisc.tile([128, 512], F32, name="pm")
                for b in range(4):
                    gblk = sb * 4 + b
                    nc.tensor.matmul(
                        plg[:, b * NACT : (b + 1) * NACT],
                        nbT_sb[:, gblk * 128 : (gblk + 1) * 128],
                        wout_sb[:],
                        start=True,
                        stop=False,
                    )
                    nc.tensor.matmul(
                        plg[:, b * NACT : (b + 1) * NACT],
                        ones_sb[:, 0:H],
                        brow_sb[:, BOUT_O : BOUT_O + NACT],
                        start=False,
                        stop=True,
                    )
                nc.vector.tensor_copy(
                    lg_sb[:, sb * 4 * NACT : (sb + 1) * 4 * NACT],
                    plg[:, 0 : 4 * NACT],
                )

            nc.sync.dma_start(out_nbT_d[:], nbT_sb[:])
            nc.sync.dma_start(out_lg_d[:], lg_sb[:])

    nc.compile()
    return nc


_NC_CACHE = {}


def _get_nc():
    if "nc" not in _NC_CACHE:
        _NC_CACHE["nc"] = _build_nc()
    return _NC_CACHE["nc"]


def _prep_inputs(inputs):
    f32 = np.float32
    obs = np.asarray(inputs["obs"], f32)
    beliefs = np.asarray(inputs["beliefs"], f32)
    W1 = np.asarray(inputs["W1"], f32)
    b1 = np.asarray(inputs["b1"], f32)
    W2 = np.asarray(inputs["W2"], f32)
    b2 = np.asarray(inputs["b2"], f32)
    Wb = np.asarray(inputs["Wb"], f32)
    bb = np.asarray(inputs["bb"], f32)
    Wg = np.asarray(inputs["Wg"], f32)
    bg = np.asarray(inputs["bg"], f32)
    Wir = np.asarray(inputs["Wir"], f32)
    Wiz = np.asarray(inputs["Wiz"], f32)
    Win = np.asarray(inputs["Win"], f32)
    Whr = np.asarray(inputs["Whr"], f32)
    bhr = np.asarray(inputs["bhr"], f32)
    Whz = np.asarray(inputs["Whz"], f32)
    bhz = np.asarray(inputs["bhz"], f32)
    Whn = np.asarray(inputs["Whn"], f32)
    bhn = np.asarray(inputs["bhn"], f32)
    Wout = np.asarray(inputs["Wout"], f32)
    bout = np.asarray(inputs["bout"], f32)

    # composite GRU input weights (gru_in never materializes on device)
    Wgr = Wg @ Wir
    Wgz = Wg @ Wiz
    Wgn = Wg @ Win
    bgr = bg @ Wir
    bgz = bg @ Wiz
    bgn = bg @ Win

    # wpack [128, WCOLS] bf16 (belief slice appended per core below)
    wcommon = np.zeros((128, BELB_O), BF16_NP)
    W1p = np.zeros((OBSP, E), f32)
    W1p[:OBS] = W1
    wcommon[:, W1_O : W1_O + KC * E] = (
        W1p.reshape(KC, 128, E).transpose(1, 0, 2).reshape(128, KC * E).astype(BF16_NP)
    )
    wcommon[0:E, W2_O : W2_O + E] = W2.astype(BF16_NP)
    wcommon[:, WB_O : WB_O + E] = Wb.astype(BF16_NP)
    wcommon[:, WGR_O : WGR_O + H] = Wgr.astype(BF16_NP)
    wcommon[:, WGZ_O : WGZ_O + H] = Wgz.astype(BF16_NP)
    wcommon[:, WGN_O : WGN_O + H] = Wgn.astype(BF16_NP)
    wcommon[:, WHR_O : WHR_O + H] = Whr.astype(BF16_NP)
    wcommon[:, WHZ_O : WHZ_O + H] = Whz.astype(BF16_NP)
    wcommon[:, WHN_O : WHN_O + H] = (0.5 * Whn).astype(BF16_NP)
    wcommon[:, ID_O : ID_O + H] = np.eye(128, dtype=BF16_NP)

    brow = np.zeros((1, BCOLS), BF16_NP)
    brow[0, B1_O : B1_O + E] = b1.astype(BF16_NP)
    brow[0, B2_O : B2_O + E] = b2.astype(BF16_NP)
    brow[0, BB_O : BB_O + E] = bb.astype(BF16_NP)
    brow[0, BR_O : BR_O + H] = (bhr + bgr).astype(BF16_NP)
    brow[0, BZ_O : BZ_O + H] = (bhz + bgz).astype(BF16_NP)
    brow[0, BHN_O : BHN_O + H] = (0.5 * bhn).astype(BF16_NP)
    brow[0, BGN_O : BGN_O + H] = bgn.astype(BF16_NP)
    brow[0, BOUT_O : BOUT_O + NACT] = bout.astype(BF16_NP)

    wout = np.ascontiguousarray(Wout, f32)

    obsTp = np.zeros((OBSP, A), BF16_NP)
    obsTp[:OBS] = obs.T.astype(BF16_NP)
    belT = np.ascontiguousarray(beliefs.T, f32)
    belTb = belT.astype(BF16_NP)

    in_maps = []
    for c in range(CORES):
        asl = slice(c * AL, (c + 1) * AL)
        wpack = np.concatenate([wcommon, belTb[:, asl]], axis=1)
        obsT_c = np.ascontiguousarray(
            obsTp[:, asl].reshape(KC, 128, AL).transpose(1, 0, 2).reshape(128, KC * AL)
        )
        in_maps.append(
            {
                "obsT": obsT_c,
                "belT": np.ascontiguousarray(belT[:, asl]),
                "wpack": np.ascontiguousarray(wpack),
                "brow": brow,
                "wout": wout,
            }
        )
    return in_maps


def run_sharded(inputs, trace=False, **kw):
    """Run the device kernel; returns (logits, new_beliefs, BassKernelResults)."""
    nc = _get_nc()
    in_maps = _prep_inputs(inputs)
    res = run_bass_kernel_spmd(
        nc, in_maps, core_ids=list(range(CORES)), trace=trace, **kw
    )
    logits = np.empty((A, NACT), np.float32)
    new_beliefs = np.empty((A, H), np.float32)
    for c, r in enumerate(res.results):
        asl = slice(c * AL, (c + 1) * AL)
        new_beliefs[asl] = r["out_nbT"].T
        logits[asl] = (
            r["out_lg"].reshape(128, AL // 128, NACT).transpose(1, 0, 2).reshape(AL, NACT)
        )
    return logits, new_beliefs, res


def kernel(**inputs):
    logits, new_beliefs, _ = run_sharded(inputs)
    return logits, new_beliefs


# revision 3
# speedup vs baseline: 1.8286x; 1.8286x over previous
"""AERIALAgent distributed Trainium2 kernel (8 NeuronCores).

Strategy (row/data parallel over the agent axis):
  - Each core owns 1024 of the 8192 agents: obs MLP, attention rows, GRU
    and outputs for its slice.
  - The belief projection bp = beliefs @ Wb + bb (8192x64, 134 MFLOP ~ 0.7%
    of total work) is computed once on the host in bf16 and REPLICATED to
    all cores in the layouts the attention needs (bpT, bp_aug tiles). In
    this environment a 1MB AllGather costs 60-90us (ncfw rendezvous barrier
    + RDH), dwarfing the projection itself, so replication beats the
    collective by a wide margin and removes all cross-core dependencies.
  - The 8192x8192 attention is flash-style per core with TRANSPOSED score
    tiles ST[j, a] so the softmax-weighted accumulation needs no on-device
    transposes. QK pairs run concurrently on the PE via row tiling
    (tile_position (0,0)/(64,0), K=64 each).
  - Softmax runs without max subtraction (logits in [-3, 6]); the diagonal
    mask is applied algebraically by subtracting exp(|bp_a|^2/8) terms from
    the context numerator and denominator after accumulation.
  - GRU runs fully in transposed [hidden, agent] layout with composite
    weights (Wg @ Wi*) folded on the host; sigmoid is computed via tanh so
    one ACT table set (exp/tanh/relu/copy) serves the whole kernel.
  - Matmuls are bf16 (f32 PSUM) except the tiny logits head / broadcast
    helpers. Outputs return transposed; the host transposes back.
"""

import numpy as np
import ml_dtypes

import concourse.bass as bass
import concourse.bacc as bacc
import concourse.mybir as mybir
import concourse.tile as tile
from concourse.bass_utils import run_bass_kernel_spmd

BF16_NP = ml_dtypes.bfloat16
F32 = mybir.dt.float32
BF16 = mybir.dt.bfloat16
AF = mybir.ActivationFunctionType
ALU = mybir.AluOpType

CORES = 8
A = 8192           # total agents
AL = A // CORES    # 1024 agents per core
OBS = 520
OBSP = 640         # obs dim padded to 5 * 128
KC = OBSP // 128   # 5 contraction chunks for the obs matmul
E = 64             # embed
H = 128            # hidden
NACT = 6
JT = A // 128      # 64 key tiles
NSB = AL // 512    # 2 superblocks of 512 query agents
JSTR = 66          # bp_aug per-tile column stride (64 data + 1 ones + 1 pad)
NBPT = 4           # bpT2 / bp_aug split into 4 tiles for DMA/compute overlap

# wpack column offsets (bf16 [128, WCOLS])
W1_O = 0              # 5 chunks of 64
W2_O = 320
WGR_O = 384
WGZ_O = 512
WGN_O = 640
WHR_O = 768
WHZ_O = 896
WHN_O = 1024          # 0.5 * Whn
BELB_O = 1152         # own beliefs.T bf16 [128, 1024]
WCOLS = BELB_O + AL

# brow column offsets (bf16 [1, BCOLS])
B1_O = 0
B2_O = 64
BR_O = 128            # bhr + bg @ Wir
BZ_O = 256            # bhz + bg @ Wiz
BHN_O = 384           # 0.5 * bhn
BGN_O = 512           # bg @ Win
BOUT_O = 640
BCOLS = 648


def _build_nc():
    nc = bacc.Bacc(
        "TRN2",
        target_bir_lowering=False,
        debug=False,
        num_devices=CORES,
    )

    obsT_d = nc.dram_tensor("obsT", [128, KC * AL], BF16, kind="ExternalInput")
    belT_d = nc.dram_tensor("belT", [H, AL], F32, kind="ExternalInput")
    wpack_d = nc.dram_tensor("wpack", [128, WCOLS], BF16, kind="ExternalInput")
    brow_d = nc.dram_tensor("brow", [1, BCOLS], BF16, kind="ExternalInput")
    wout_d = nc.dram_tensor("wout", [H, NACT], F32, kind="ExternalInput")
    bpT2_d = nc.dram_tensor("bpT2", [128, A], BF16, kind="ExternalInput")
    bpaug_d = nc.dram_tensor("bpaug", [128, JT * JSTR], BF16, kind="ExternalInput")
    bpTl2_d = nc.dram_tensor("bpTl2", [128, AL], BF16, kind="ExternalInput")
    expd_d = nc.dram_tensor("expd", [1, AL], F32, kind="ExternalInput")

    out_nbT_d = nc.dram_tensor("out_nbT", [H, AL], F32, kind="ExternalOutput")
    out_lg_d = nc.dram_tensor(
        "out_lg", [128, (AL // 128) * NACT], F32, kind="ExternalOutput"
    )

    JPT = JT // NBPT  # key tiles per bpT2/bpaug tile

    with tile.TileContext(nc) as tc:
        with (
            tc.tile_pool(name="const", bufs=1) as const,
            tc.tile_pool(name="work", bufs=2) as work,
            tc.tile_pool(name="expp", bufs=4) as expp,
            tc.tile_pool(name="pst", bufs=2, space=bass.MemorySpace.PSUM) as pst,
            tc.tile_pool(name="pctx", bufs=2, space=bass.MemorySpace.PSUM) as pctx,
            tc.tile_pool(name="pmisc", bufs=2, space=bass.MemorySpace.PSUM) as pmisc,
        ):
            # ---------------- inputs ----------------
            bpTl2_sb = const.tile([128, AL], BF16, name="bpTl2_sb")
            nc.sync.dma_start(bpTl2_sb[:], bpTl2_d[:])
            bpT2_sb = []
            bpaug_sb = []
            for k in range(NBPT):
                t = const.tile([128, JPT * 128], BF16, name=f"bpT2_{k}")
                nc.sync.dma_start(t[:], bpT2_d[:, k * JPT * 128 : (k + 1) * JPT * 128])
                bpT2_sb.append(t)
            for k in range(NBPT):
                t = const.tile([128, JPT * JSTR], BF16, name=f"bpaug_{k}")
                nc.sync.dma_start(
                    t[:], bpaug_d[:, k * JPT * JSTR : (k + 1) * JPT * JSTR]
                )
                bpaug_sb.append(t)

            w_sb = const.tile([128, WCOLS], BF16, name="w_sb")
            obsT_sb = const.tile([128, KC * AL], BF16, name="obsT_sb")
            belT_sb = const.tile([H, AL], F32, name="belT_sb")
            brow_sb = const.tile([1, BCOLS], BF16, name="brow_sb")
            wout_sb = const.tile([H, NACT], F32, name="wout_sb")
            expd_sb = const.tile([1, AL], F32, name="expd_sb")

            nc.sync.dma_start(w_sb[:], wpack_d[:])
            nc.sync.dma_start(obsT_sb[:], obsT_d[:])
            nc.sync.dma_start(belT_sb[:], belT_d[:])
            nc.sync.dma_start(brow_sb[:], brow_d[:])
            nc.sync.dma_start(wout_sb[:], wout_d[:])
            nc.sync.dma_start(expd_sb[:], expd_d[:])

            ones_sb = const.tile([1, 512], BF16, name="ones_sb")
            onesf_sb = const.tile([1, E], F32, name="onesf_sb")
            nc.vector.memset(ones_sb[:], 1.0)
            nc.vector.memset(onesf_sb[:], 1.0)

            h1T_sb = const.tile([E, AL], BF16, name="h1T_sb")
            concatT_sb = const.tile([H, AL], BF16, name="concatT_sb")
            nbT_sb = const.tile([H, AL], F32, name="nbT_sb")
            lg_sb = const.tile([128, (AL // 128) * NACT], F32, name="lg_sb")
            den_sb = const.tile([1, AL], F32, name="den_sb")
            recip_sb = const.tile([1, AL], F32, name="recip_sb")

            # -------- obs MLP (transposed): h2T = relu(W2.T relu(W1.T obsT + b1) + b2) --------
            for ch in range(2):
                cs = slice(ch * 512, (ch + 1) * 512)
                p1 = pmisc.tile([128, 512], F32, name="pm")
                for kc in range(KC):
                    nc.tensor.matmul(
                        p1[0:E, :],
                        w_sb[:, W1_O + kc * E : W1_O + (kc + 1) * E],
                        obsT_sb[:, kc * AL + ch * 512 : kc * AL + (ch + 1) * 512],
                        start=(kc == 0),
                        stop=False,
                    )
                nc.tensor.matmul(
                    p1[0:E, :],
                    brow_sb[:, B1_O : B1_O + E],
                    ones_sb[:, 0:512],
                    start=False,
                    stop=True,
                )
                nc.scalar.activation(h1T_sb[:, cs], p1[0:E, :], AF.Relu)
            for ch in range(2):
                cs = slice(ch * 512, (ch + 1) * 512)
                p2 = pmisc.tile([128, 512], F32, name="pm")
                nc.tensor.matmul(
                    p2[0:E, :],
                    w_sb[0:E, W2_O : W2_O + E],
                    h1T_sb[:, cs],
                    start=True,
                    stop=False,
                )
                nc.tensor.matmul(
                    p2[0:E, :],
                    brow_sb[:, B2_O : B2_O + E],
                    ones_sb[:, 0:512],
                    start=False,
                    stop=True,
                )
                nc.scalar.activation(concatT_sb[0:E, cs], p2[0:E, :], AF.Relu)

            # -------- attention + context fix + GRU, per 512-agent superblock --------
            for sb in range(NSB):
                cs = slice(sb * 512, (sb + 1) * 512)
                ctx = pctx.tile([128, 512], F32, name="ctx")
                for g in range(JT // 2):
                    jt0, jt1 = 2 * g, 2 * g + 1
                    st = pst.tile([128, 1024], F32, name="st")
                    # two K=64 QK matmuls run concurrently in disjoint PE row groups
                    nc.tensor.matmul(
                        st[:, 0:512],
                        bpT2_sb[jt0 // JPT][0:E, (jt0 % JPT) * 128 : (jt0 % JPT + 1) * 128],
                        bpTl2_sb[0:E, cs],
                        start=True,
                        stop=True,
                        tile_position=(0, 0),
                    )
                    nc.tensor.matmul(
                        st[:, 512:1024],
                        bpT2_sb[jt1 // JPT][E:128, (jt1 % JPT) * 128 : (jt1 % JPT + 1) * 128],
                        bpTl2_sb[E:128, cs],
                        start=True,
                        stop=True,
                        tile_position=(64, 0),
                    )
                    ex = expp.tile([128, 1024], BF16, name="ex")
                    nc.scalar.activation(ex[:], st[:], AF.Exp, scale=0.125)
                    for i, jt in ((0, jt0), (1, jt1)):
                        nc.tensor.matmul(
                            ctx[0 : E + 1, :],
                            bpaug_sb[jt // JPT][
                                :, (jt % JPT) * JSTR : (jt % JPT) * JSTR + E + 1
                            ],
                            ex[:, i * 512 : (i + 1) * 512],
                            start=(jt == 0),
                            stop=(jt == JT - 1),
                        )

                # remove diagonal; normalize; write context into concatT rows 64..127
                nc.vector.tensor_tensor(
                    den_sb[:, cs], ctx[E : E + 1, :], expd_sb[:, cs], ALU.subtract
                )
                nc.vector.reciprocal(recip_sb[:, cs], den_sb[:, cs])
                edb = pmisc.tile([128, 512], F32, name="pm")
                nc.tensor.matmul(
                    edb[0:E, :], onesf_sb[:], expd_sb[:, cs], start=True, stop=True
                )
                rb = pmisc.tile([128, 512], F32, name="pm")
                nc.tensor.matmul(
                    rb[0:E, :], onesf_sb[:], recip_sb[:, cs], start=True, stop=True
                )
                m_sb = work.tile([E, 512], F32, name="m_sb")
                nc.vector.tensor_tensor(m_sb[:], bpTl2_sb[0:E, cs], edb[0:E, :], ALU.mult)
                t_sb = work.tile([E, 512], F32, name="t_sb")
                nc.vector.tensor_tensor(t_sb[:], ctx[0:E, :], m_sb[:], ALU.subtract)
                nc.vector.tensor_tensor(
                    concatT_sb[E:H, cs], t_sb[:], rb[0:E, :], ALU.mult
                )

                # ---- GRU for this 512-agent chunk (transposed layout) ----
                belb_rhs = w_sb[:, BELB_O + sb * 512 : BELB_O + (sb + 1) * 512]

                pr = pmisc.tile([128, 512], F32, name="pm")
                nc.tensor.matmul(pr[:], w_sb[:, WGR_O : WGR_O + H], concatT_sb[:, cs],
                                 start=True, stop=False)
                nc.tensor.matmul(pr[:], w_sb[:, WHR_O : WHR_O + H], belb_rhs,
                                 start=False, stop=False)
                nc.tensor.matmul(pr[:], brow_sb[:, BR_O : BR_O + H], ones_sb[:, 0:512],
                                 start=False, stop=True)
                tr_sb = work.tile([H, 512], F32, name="tr_sb")
                nc.scalar.activation(tr_sb[:], pr[:], AF.Tanh, scale=0.5)

                pz = pmisc.tile([128, 512], F32, name="pm")
                nc.tensor.matmul(pz[:], w_sb[:, WGZ_O : WGZ_O + H], concatT_sb[:, cs],
                                 start=True, stop=False)
                nc.tensor.matmul(pz[:], w_sb[:, WHZ_O : WHZ_O + H], belb_rhs,
                                 start=False, stop=False)
                nc.tensor.matmul(pz[:], brow_sb[:, BZ_O : BZ_O + H], ones_sb[:, 0:512],
                                 start=False, stop=True)
                tz_sb = work.tile([H, 512], F32, name="tz_sb")
                nc.scalar.activation(tz_sb[:], pz[:], AF.Tanh, scale=0.5)

                pu = pmisc.tile([128, 512], F32, name="pm")
                nc.tensor.matmul(pu[:], w_sb[:, WHN_O : WHN_O + H], belb_rhs,
                                 start=True, stop=False)
                nc.tensor.matmul(pu[:], brow_sb[:, BHN_O : BHN_O + H], ones_sb[:, 0:512],
                                 start=False, stop=True)
                pn = pmisc.tile([128, 512], F32, name="pm")
                nc.tensor.matmul(pn[:], w_sb[:, WGN_O : WGN_O + H], concatT_sb[:, cs],
                                 start=True, stop=False)
                nc.tensor.matmul(pn[:], brow_sb[:, BGN_O : BGN_O + H], ones_sb[:, 0:512],
                                 start=False, stop=True)
                # v = (tr + 1) * u'   (u' = 0.5*(bel@Whn + bhn))
                v_sb = work.tile([H, 512], F32, name="v_sb")
                nc.vector.scalar_tensor_tensor(
                    v_sb[:], tr_sb[:], 1.0, pu[:], ALU.add, ALU.mult
                )
                t2_sb = work.tile([H, 512], F32, name="t2_sb")
                nc.vector.tensor_tensor(t2_sb[:], v_sb[:], pn[:], ALU.add)
                n_sb = work.tile([H, 512], F32, name="n_sb")
                nc.scalar.activation(n_sb[:], t2_sb[:], AF.Tanh)

                # new_beliefs = n + 0.5*(tz + 1)*(beliefs - n)
                d_sb = work.tile([H, 512], F32, name="d_sb")
                nc.vector.tensor_tensor(d_sb[:], belT_sb[:, cs], n_sb[:], ALU.subtract)
                w2_sb = work.tile([H, 512], F32, name="w2_sb")
                nc.vector.scalar_tensor_tensor(
                    w2_sb[:], tz_sb[:], 1.0, d_sb[:], ALU.add, ALU.mult
                )
                nc.vector.scalar_tensor_tensor(
                    nbT_sb[:, cs], w2_sb[:], 0.5, n_sb[:], ALU.mult, ALU.add
                )

                # logits for the 4 query blocks in this chunk
                plg = pmisc.tile([128, 512], F32, name="pm")
                for b in range(4):
                    gblk = sb * 4 + b
                    nc.tensor.matmul(
                        plg[:, b * NACT : (b + 1) * NACT],
                        nbT_sb[:, gblk * 128 : (gblk + 1) * 128],
                        wout_sb[:],
                        start=True,
                        stop=False,
                    )
                    nc.tensor.matmul(
                        plg[:, b * NACT : (b + 1) * NACT],
                        ones_sb[:, 0:H],
                        brow_sb[:, BOUT_O : BOUT_O + NACT],
                        start=False,
                        stop=True,
                    )
                nc.vector.tensor_copy(
                    lg_sb[:, sb * 4 * NACT : (sb + 1) * 4 * NACT],
                    plg[:, 0 : 4 * NACT],
                )

            nc.sync.dma_start(out_nbT_d[:], nbT_sb[:])
            nc.sync.dma_start(out_lg_d[:], lg_sb[:])

    nc.compile()
    return nc


_NC_CACHE = {}


def _get_nc():
    if "nc" not in _NC_CACHE:
        _NC_CACHE["nc"] = _build_nc()
    return _NC_CACHE["nc"]


def _prep_inputs(inputs):
    f32 = np.float32
    obs = np.asarray(inputs["obs"], f32)
    beliefs = np.asarray(inputs["beliefs"], f32)
    W1 = np.asarray(inputs["W1"], f32)
    b1 = np.asarray(inputs["b1"], f32)
    W2 = np.asarray(inputs["W2"], f32)
    b2 = np.asarray(inputs["b2"], f32)
    Wb = np.asarray(inputs["Wb"], f32)
    bb = np.asarray(inputs["bb"], f32)
    Wg = np.asarray(inputs["Wg"], f32)
    bg = np.asarray(inputs["bg"], f32)
    Wir = np.asarray(inputs["Wir"], f32)
    Wiz = np.asarray(inputs["Wiz"], f32)
    Win = np.asarray(inputs["Win"], f32)
    Whr = np.asarray(inputs["Whr"], f32)
    bhr = np.asarray(inputs["bhr"], f32)
    Whz = np.asarray(inputs["Whz"], f32)
    bhz = np.asarray(inputs["bhz"], f32)
    Whn = np.asarray(inputs["Whn"], f32)
    bhn = np.asarray(inputs["bhn"], f32)
    Wout = np.asarray(inputs["Wout"], f32)
    bout = np.asarray(inputs["bout"], f32)

    # composite GRU input weights (gru_in never materializes on device)
    Wgr = Wg @ Wir
    Wgz = Wg @ Wiz
    Wgn = Wg @ Win
    bgr = bg @ Wir
    bgz = bg @ Wiz
    bgn = bg @ Win

    wcommon = np.zeros((128, BELB_O), BF16_NP)
    W1p = np.zeros((OBSP, E), f32)
    W1p[:OBS] = W1
    wcommon[:, W1_O : W1_O + KC * E] = (
        W1p.reshape(KC, 128, E).transpose(1, 0, 2).reshape(128, KC * E).astype(BF16_NP)
    )
    wcommon[0:E, W2_O : W2_O + E] = W2.astype(BF16_NP)
    wcommon[:, WGR_O : WGR_O + H] = Wgr.astype(BF16_NP)
    wcommon[:, WGZ_O : WGZ_O + H] = Wgz.astype(BF16_NP)
    wcommon[:, WGN_O : WGN_O + H] = Wgn.astype(BF16_NP)
    wcommon[:, WHR_O : WHR_O + H] = Whr.astype(BF16_NP)
    wcommon[:, WHZ_O : WHZ_O + H] = Whz.astype(BF16_NP)
    wcommon[:, WHN_O : WHN_O + H] = (0.5 * Whn).astype(BF16_NP)

    brow = np.zeros((1, BCOLS), BF16_NP)
    brow[0, B1_O : B1_O + E] = b1.astype(BF16_NP)
    brow[0, B2_O : B2_O + E] = b2.astype(BF16_NP)
    brow[0, BR_O : BR_O + H] = (bhr + bgr).astype(BF16_NP)
    brow[0, BZ_O : BZ_O + H] = (bhz + bgz).astype(BF16_NP)
    brow[0, BHN_O : BHN_O + H] = (0.5 * bhn).astype(BF16_NP)
    brow[0, BGN_O : BGN_O + H] = bgn.astype(BF16_NP)
    brow[0, BOUT_O : BOUT_O + NACT] = bout.astype(BF16_NP)

    wout = np.ascontiguousarray(Wout, f32)

    obsTp = np.zeros((OBSP, A), BF16_NP)
    obsTp[:OBS] = obs.T.astype(BF16_NP)
    belT = np.ascontiguousarray(beliefs.T, f32)
    belTb = belT.astype(BF16_NP)

    # host belief projection, bf16-rounded exactly as the device would
    bp = (beliefs.astype(BF16_NP).astype(f32) @ Wb.astype(BF16_NP).astype(f32) + bb)
    bp16 = bp.astype(BF16_NP)                      # [A, E]
    bpT16 = np.ascontiguousarray(bp16.T)           # [E, A]
    bpT2 = np.concatenate([bpT16, bpT16], axis=0)  # [128, A] duplicated halves
    # bp_aug: per key tile [128, 66] = [bp rows | ones | pad]
    bpaug = np.zeros((128, JT * JSTR), BF16_NP)
    bpaug3 = bpaug.reshape(128, JT, JSTR)
    bpaug3[:, :, 0:E] = bp16.reshape(JT, 128, E).transpose(1, 0, 2)
    bpaug3[:, :, E] = 1.0
    # diag correction terms: exp(|bp_a|^2 / 8) in f32 from the bf16 bp
    selfdot = (bp16.astype(f32) ** 2).sum(axis=1)
    expd = np.exp(0.125 * selfdot).astype(f32)[None, :]  # [1, A]

    in_maps = []
    for c in range(CORES):
        asl = slice(c * AL, (c + 1) * AL)
        wpack = np.concatenate([wcommon, belTb[:, asl]], axis=1)
        obsT_c = np.ascontiguousarray(
            obsTp[:, asl].reshape(KC, 128, AL).transpose(1, 0, 2).reshape(128, KC * AL)
        )
        bpTl = bpT16[:, asl]
        bpTl2 = np.concatenate([bpTl, bpTl], axis=0)  # [128, AL]
        in_maps.append(
            {
                "obsT": obsT_c,
                "belT": np.ascontiguousarray(belT[:, asl]),
                "wpack": np.ascontiguousarray(wpack),
                "brow": brow,
                "wout": wout,
                "bpT2": bpT2,
                "bpaug": bpaug,
                "bpTl2": np.ascontiguousarray(bpTl2),
                "expd": np.ascontiguousarray(expd[:, asl]),
            }
        )
    return in_maps


def run_sharded(inputs, trace=False, **kw):
    """Run the device kernel; returns (logits, new_beliefs, BassKernelResults)."""
    nc = _get_nc()
    in_maps = _prep_inputs(inputs)
    res = run_bass_kernel_spmd(
        nc, in_maps, core_ids=list(range(CORES)), trace=trace, **kw
    )
    logits = np.empty((A, NACT), np.float32)
    new_beliefs = np.empty((A, H), np.float32)
    for c, r in enumerate(res.results):
        asl = slice(c * AL, (c + 1) * AL)
        new_beliefs[asl] = r["out_nbT"].T
        logits[asl] = (
            r["out_lg"].reshape(128, AL // 128, NACT).transpose(1, 0, 2).reshape(AL, NACT)
        )
    return logits, new_beliefs, res


def kernel(**inputs):
    logits, new_beliefs, _ = run_sharded(inputs)
    return logits, new_beliefs
